# revision 12
# baseline (speedup 1.0000x reference)
"""Linear-attention (relu, rmsnorm-qk) Trainium2 Bass kernel, 8 NeuronCores.

Sharding: each core owns 1/4 of the tokens of TWO batch elements:
  cores 0-3 -> batches 0 (group g=0) and 1 (g=1)
  cores 4-7 -> batches 2 (g=0) and 3 (g=1)
Within a batch, core q (= core_id % 4) owns tokens [1024*q, 1024*(q+1)).

v3 design notes (fp8 DoubleRow with hi+lo error compensation):
 - The two big GEMMs (qkv projection, output projection) run as fp8e4m3
   DoubleRow matmuls (2 contraction rows per partition, 0.5 cyc/row).
   Full accuracy is kept by splitting each operand X into
   X_hi = fp8(X), X_lo = fp8(X - X_hi) and accumulating three passes
   hi@hi + lo@hi + hi@lo in PSUM (the dropped lo@lo term is ~0.1%).
   Net cost 6 cyc/col vs bf16's 8, with bf16-level accuracy.
 - W_qkv/W_out are pre-scaled by 32 (sigma 1) so fp8 normals cover them;
   the 32s cancel between attention numerator/denominator, and the
   output projection applies 1/(32*alpha) at the final copy.
 - attn is produced at alpha=128 times its true scale (folded into the
   v copy-out scale) so its fp8 hi/lo split lands in e4m3's normal
   range; split runs on the otherwise-idle Pool engine.
 - q's rmsnorm scale cancels entirely (relu(s*q) = s*relu(q)); k's
   scale is folded into v and the appended ones column, with stats
   taken on the 32-scaled k (eps' = 1024*eps).
 - kv_ext = k~^T @ [v~ | s] per head-pair in PSUM over 2-tile batches,
   AllReduce over the 4 cores of the batch overlaps the other group's
   phase 1; phase 2 attn^T = blockdiag(kv)^T @ q^T stays bf16 (K=128
   per pair: DoubleRow would not help).
"""

import os
import sys

import numpy as np

for _p in ("/opt/trn_rl_repo",):
    if _p not in sys.path and os.path.isdir(_p):
        sys.path.insert(0, _p)

import concourse.mybir as mybir
import concourse.tile as tile
from concourse import bacc
from concourse.bass_utils import run_bass_kernel_spmd
from contextlib import ExitStack

F32 = mybir.dt.float32
BF16 = mybir.dt.bfloat16
FP8 = mybir.dt.float8e4
DRMODE = mybir.MatmulPerfMode.DoubleRow
ALU = mybir.AluOpType
ACTF = mybir.ActivationFunctionType

DIM = 1024
HEADS = 16
DHEAD = 64
NPAIR = HEADS // 2          # 8 head pairs
B = 4
N = 4096
TOK = 2048                  # tokens per core (2 groups x 1024)
GTOK = 1024                 # tokens per group
NTG = GTOK // 128           # 8 token tiles per group
EPS_NORM = 1e-6
EPSP = 1024.0 * EPS_NORM    # stats run on 32x-scaled k
ALPHA = 128.0               # attn pre-scale for fp8 range
OSCALE = 1.0 / (32.0 * ALPHA)
KVW = 2 * (DHEAD + 1)       # 130: kv_ext width per pair
RG = [[0, 1, 2, 3], [4, 5, 6, 7]]

_CACHE: dict = {}


def _build(use_bias: bool, use_w: bool, sim_mode: bool = False):
    ndev = 1 if sim_mode else 8
    nc = bacc.Bacc("TRN2", target_bir_lowering=False, debug=False, num_devices=ndev)

    # x^T per token tile: [t, p, hl, c', n] = hi/lo of x[tok 128t+n, dim 128c'+p]
    xT_d = nc.dram_tensor("xT", [16, 128, 2, 8, 128], FP8, kind="ExternalInput").ap()
    # W_qkv*32 packed for DoubleRow: [hl, c, p, i, m] = 32*W[256c+128i+p, m]
    wqkv_d = nc.dram_tensor(
        "wqkv", [2, 4, 128, 2, 3 * DIM], FP8, kind="ExternalInput"
    ).ap()
    # W_out*32 packed: [hl, p, c4, i, o] = 32*Wout[256c4+128i+p, o]
    wout_d = nc.dram_tensor(
        "wout", [2, 128, 4, 2, DIM], FP8, kind="ExternalInput"
    ).ap()
    qn_d = nc.dram_tensor("qn", [128, 8], F32, kind="ExternalInput").ap()
    kn_d = nc.dram_tensor("kn", [128, DIM], F32, kind="ExternalInput").ap()
    bout_d = nc.dram_tensor("bout", [128, DIM], F32, kind="ExternalInput").ap()
    out_d = nc.dram_tensor("out", [TOK, DIM], F32, kind="ExternalOutput").ap()

    with tile.TileContext(nc) as tc:
        with ExitStack() as outer:
            const = outer.enter_context(tc.tile_pool(name="const", bufs=1))
            wpool = outer.enter_context(tc.tile_pool(name="wpool", bufs=1))
            qTpool = outer.enter_context(tc.tile_pool(name="qTpool", bufs=1))
            stats = outer.enter_context(tc.tile_pool(name="stats", bufs=3))
            drampool = outer.enter_context(
                tc.tile_pool(name="dram", bufs=1, space="DRAM")
            )

            eps_sb = const.tile([128, 1], F32, name="eps_sb")
            nc.vector.memset(eps_sb[:], EPSP)
            eps16_sb = const.tile([128, 1], F32, name="eps16_sb")
            nc.vector.memset(eps16_sb[:], EPSP / 16.0)
            ones_sb = const.tile([128, 64], F32, name="ones_sb")
            nc.vector.memset(ones_sb[:], 1.0)
            if use_w:
                qn_sb = const.tile([128, 8], F32, name="qn_sb")
                kn_sb = const.tile([128, DIM], F32, name="kn_sb")
                nc.sync.dma_start(qn_sb[:], qn_d[:])
                nc.sync.dma_start(kn_sb[:], kn_d[:])
            if use_bias:
                bout_sb = const.tile([128, DIM], F32, name="bout_sb")
                nc.sync.dma_start(bout_sb[:], bout_d[:])
                osc_sb = const.tile([128, 1], F32, name="osc_sb")
                nc.vector.memset(osc_sb[:], OSCALE)

            # x tiles stream on the ACT queue; tile 0 is issued before W so
            # its transfer leads the serial DMA stream.
            xTp = outer.enter_context(tc.tile_pool(name="xTp", bufs=5))
            xq = {}

            def issue_x(t):
                xt = xTp.tile([128, 2, 8, 128], FP8, name=f"xT_{t}", tag="xT")
                nc.scalar.dma_start(xt[:], xT_d[t, :, :, :, :])
                xq[t] = xt

            issue_x(0)

            # W_qkv resident as fp8 hi/lo 256-dim chunks. DMA engine pool
            # drains roughly in issue order: q columns (hi then lo, consumed
            # by tile 0's q^T chains) before the k/v columns.
            w8 = [[None] * 4 for _ in range(2)]
            for hl in range(2):
                for c in range(4):
                    w = wpool.tile(
                        [128, 2, 3 * DIM], FP8, name=f"w8_{hl}_{c}",
                        tag=f"w{hl}{c}",
                    )
                    w8[hl][c] = w
                    nc.sync.dma_start(
                        w[:, :, 0:DIM], wqkv_d[hl, c, :, :, 0:DIM]
                    )
            # k then v slabs, hi before lo: exact first-use order so the
            # PE consumes each 256-dim chunk the moment it lands.
            for base in (DIM, 2 * DIM):
                for hl in range(2):
                    for c in range(4):
                        nc.sync.dma_start(
                            w8[hl][c][:, :, base : base + DIM],
                            wqkv_d[hl, c, :, :, base : base + DIM],
                        )

            # all 8 pairs' q^T in one tile: [128, pair, TOK]
            qT = qTpool.tile([128, NPAIR, TOK], BF16, name="qT", tag="qT")
            prep = outer.enter_context(tc.tile_pool(name="prep", bufs=1))

            def kv_prep(g, arout):
                # Entirely on the gpsimd queue: it is otherwise idle-ish, its
                # FIFO orders the load after the collective.
                kv_sb = prep.tile(
                    [128, NPAIR, KVW], F32, name=f"kvsb{g}", tag=f"kvsb{g}"
                )
                nc.gpsimd.dma_start(kv_sb[:], arout[:])
                bd = prep.tile([128, NPAIR, 128], BF16, name=f"bd{g}", tag=f"bd{g}")
                nc.gpsimd.memset(bd[:], 0.0)
                nc.gpsimd.tensor_copy(bd[0:64, :, 0:64], kv_sb[0:64, :, 0:64])
                nc.gpsimd.tensor_copy(
                    bd[64:128, :, 64:128], kv_sb[64:128, :, 65:129]
                )
                ksr = prep.tile(
                    [128, NPAIR, 128], BF16, name=f"ksr{g}", tag=f"ksr{g}"
                )
                nc.gpsimd.memset(ksr[:], 0.0)
                for p in range(NPAIR):
                    nc.gpsimd.tensor_scalar_mul(
                        ksr[0:64, p, 0:64], ones_sb[0:64, 0:64],
                        kv_sb[0:64, p, 64:65],
                    )
                    nc.gpsimd.tensor_scalar_mul(
                        ksr[64:128, p, 64:128], ones_sb[64:128, 0:64],
                        kv_sb[64:128, p, 129:130],
                    )
                return bd, ksr

            def dr_mm(ps, lhsT, rhs, start, stop):
                nc.tensor.matmul(
                    ps, lhsT, rhs, start=start, stop=stop, perf_mode=DRMODE
                )

            prepped = []
            kp = outer.enter_context(tc.tile_pool(name="kp", bufs=4))
            vp = outer.enter_context(tc.tile_pool(name="vp", bufs=4))
            kvpool = outer.enter_context(tc.tile_pool(name="kvpool", bufs=2))
            with ExitStack() as ph1:
                psq = ph1.enter_context(
                    tc.tile_pool(name="psq", bufs=1, space="PSUM")
                )
                psk = ph1.enter_context(
                    tc.tile_pool(name="psk", bufs=1, space="PSUM")
                )
                psv = ph1.enter_context(
                    tc.tile_pool(name="psv", bufs=1, space="PSUM")
                )
                pssm = ph1.enter_context(
                    tc.tile_pool(name="pssm", bufs=1, space="PSUM")
                )

                # DoubleRow pass list for the qkv GEMM: (w_hl, x_hl)
                PASSES = ((0, 0), (0, 1), (1, 0))

                def emit_qT(t, xt, eightbank):
                    t0r = t * 128
                    if eightbank:
                        # tile 0: all 8 banks free -> 8 chains c-major (one
                        # per bank), consuming each W q-part as it lands.
                        tags = ("q0", "q1", "k0", "k1", "v0", "v1", "sm0", "sm1")
                        pools = (psq, psq, psk, psk, psv, psv, pssm, pssm)
                        q8 = [
                            pools[j].tile(
                                [128, 128], F32, name=f"q8_{j}", tag=tags[j]
                            )
                            for j in range(8)
                        ]
                        for wh, xh in PASSES:
                            for c in range(4):
                                for j in range(8):
                                    dr_mm(
                                        q8[j][:],
                                        w8[wh][c][:, :, j * 128 : (j + 1) * 128],
                                        xt[:, xh, 2 * c : 2 * c + 2, :],
                                        start=(wh == 0 and xh == 0 and c == 0),
                                        stop=(wh == 1 and c == 3),
                                    )
                        for j in range(8):
                            nc.scalar.activation(
                                qT[:, j, t0r : t0r + 128],
                                q8[j][:],
                                ACTF.Relu,
                                scale=(qn_sb[:, j : j + 1] if use_w else 1.0),
                            )
                        return
                    qps = [
                        psq.tile(
                            [128, 4, 128], F32,
                            name=f"qps{t}_{half}", tag=f"q{half}",
                        )
                        for half in range(2)
                    ]
                    for half in range(2):
                        for jj in range(4):
                            j = 4 * half + jj
                            for pi, (wh, xh) in enumerate(PASSES):
                                for c in range(4):
                                    dr_mm(
                                        qps[half][:, jj, :],
                                        w8[wh][c][:, :, j * 128 : (j + 1) * 128],
                                        xt[:, xh, 2 * c : 2 * c + 2, :],
                                        start=(pi == 0 and c == 0),
                                        stop=(pi == 2 and c == 3),
                                    )
                        if use_w:
                            for jj in range(4):
                                j = 4 * half + jj
                                nc.scalar.activation(
                                    qT[:, j, t0r : t0r + 128],
                                    qps[half][:, jj, :],
                                    ACTF.Relu,
                                    scale=qn_sb[:, j : j + 1],
                                )
                        else:
                            nc.scalar.activation(
                                qT[:, 4 * half : 4 * half + 4, t0r : t0r + 128],
                                qps[half][:],
                                ACTF.Relu,
                                scale=1.0,
                            )

                def emit_kv_mm(t, xt):
                    kps = [
                        psk.tile([128, 512], F32, name=f"kps{t}_{h}", tag=f"k{h}")
                        for h in range(2)
                    ]
                    vps = [
                        psv.tile([128, 512], F32, name=f"vps{t}_{h}", tag=f"v{h}")
                        for h in range(2)
                    ]

                    def kv_chunk(ps, base, h, c, wh, xh, start, stop):
                        dr_mm(
                            ps[h][:],
                            xt[:, xh, 2 * c : 2 * c + 2, :],
                            w8[wh][c][
                                :, :, base + h * 512 : base + (h + 1) * 512
                            ],
                            start=start,
                            stop=stop,
                        )

                    for h in range(2):
                        for pi, (wh, xh) in enumerate(PASSES):
                            for c in range(4):
                                kv_chunk(
                                    kps, DIM, h, c, wh, xh,
                                    (pi == 0 and c == 0),
                                    (pi == 2 and c == 3),
                                )
                    for h in range(2):
                        for pi, (wh, xh) in enumerate(PASSES):
                            for c in range(4):
                                kv_chunk(
                                    vps, 2 * DIM, h, c, wh, xh,
                                    (pi == 0 and c == 0),
                                    (pi == 2 and c == 3),
                                )
                    return kps, vps

                def emit_epilogue(t, kps, vps):
                    # rmsnorm scale on 32x-scaled k: s = 1/sqrt(ms + 1024eps)
                    # (= s_true/32, the ones-column scale); v copy scale is
                    # 4x that (alpha=128 folded: alpha*s_true/1024).
                    st6 = stats.tile([128, 2, 6], F32, name=f"st6_{t}", tag="st6")
                    nc.vector.bn_stats(st6[:, 0, :], kps[0][:])
                    nc.vector.bn_stats(st6[:, 1, :], kps[1][:])
                    mv = stats.tile([128, 2], F32, name=f"mv_{t}", tag="mv")
                    nc.vector.bn_aggr(mv[:], st6[:])
                    ms = stats.tile([128, 1], F32, name=f"ms_{t}", tag="ms")
                    nc.vector.scalar_tensor_tensor(
                        out=ms[:],
                        in0=mv[:, 0:1],
                        scalar=mv[:, 0:1],
                        in1=mv[:, 1:2],
                        op0=ALU.mult,
                        op1=ALU.add,
                    )
                    a0 = stats.tile([128, 1], F32, name=f"a0_{t}", tag="a0")
                    nc.scalar.activation(
                        a0[:], ms[:], ACTF.Sqrt, bias=eps_sb[:], scale=1.0
                    )
                    s = stats.tile([128, 1], F32, name=f"s_{t}", tag="s")
                    nc.vector.reciprocal(s[:], a0[:])
                    a0v = stats.tile([128, 1], F32, name=f"a0v_{t}", tag="a0v")
                    nc.scalar.activation(
                        a0v[:], ms[:], ACTF.Sqrt, bias=eps16_sb[:], scale=0.0625
                    )
                    sv = stats.tile([128, 1], F32, name=f"sv_{t}", tag="sv")
                    nc.vector.reciprocal(sv[:], a0v[:])

                    # k~ = relu(k) (scale folded into v); general path
                    # applies kn first on DVE.
                    k_sb = kp.tile([128, DIM], BF16, name=f"ksb{t}", tag="ksb")
                    for h in range(2):
                        sl = slice(h * 512, (h + 1) * 512)
                        if use_w:
                            nc.vector.tensor_tensor(
                                k_sb[:, sl], kps[h][:], kn_sb[:, sl], ALU.mult
                            )
                            nc.scalar.activation(
                                k_sb[:, sl], k_sb[:, sl], ACTF.Relu
                            )
                        else:
                            nc.scalar.activation(
                                k_sb[:, sl], kps[h][:], ACTF.Relu
                            )

                    v_sb = vp.tile(
                        [128, HEADS, DHEAD + 1], BF16, name=f"vsb{t}", tag="vsb"
                    )
                    for h in range(2):
                        nc.scalar.activation(
                            v_sb[:, 8 * h : 8 * (h + 1), 0:DHEAD],
                            vps[h].rearrange("p (h e) -> p h e", e=DHEAD),
                            ACTF.Copy,
                            scale=sv[:],
                        )
                    nc.vector.tensor_scalar_mul(
                        v_sb[:, :, DHEAD], ones_sb[:, 0:16], s[:]
                    )
                    return k_sb, v_sb

                def emit_kvbatch(i, t, sb0, sb1, kv_acc):
                    pk, pv = sb0
                    k_sb, v_sb = sb1
                    for grp, prs in ((0, (0, 1, 2)), (1, (3, 4, 5)), (2, (6, 7))):
                        kvp = pssm.tile(
                            [128, len(prs), KVW], F32,
                            name=f"kv{t}_{grp}",
                            tag=f"sm{0 if grp != 1 else 1}",
                        )
                        for pi, p in enumerate(prs):
                            for ii, (ks_, vs_) in enumerate(
                                ((pk, pv), (k_sb, v_sb))
                            ):
                                nc.tensor.matmul(
                                    kvp[:, pi, :],
                                    ks_[:, p * 128 : (p + 1) * 128],
                                    vs_[:, 2 * p : 2 * p + 2, :],
                                    start=(ii == 0),
                                    stop=(ii == 1),
                                )
                        if i == 1:
                            nc.vector.tensor_copy(
                                kv_acc[:, prs[0] : prs[-1] + 1, :], kvp[:]
                            )
                        else:
                            nc.vector.tensor_add(
                                kv_acc[:, prs[0] : prs[-1] + 1, :],
                                kv_acc[:, prs[0] : prs[-1] + 1, :],
                                kvp[:],
                            )

                for g in range(2):
                    kv_acc = kvpool.tile(
                        [128, NPAIR, KVW], F32, name=f"kvacc{g}", tag="kvacc"
                    )
                    if g > 0:
                        issue_x(8 * g)
                    issue_x(8 * g + 1)
                    hold = None
                    start_i = 0
                    if g == 0:
                        # prologue: both tiles' q^T (gated only on the early
                        # q-part stream) run before any k/v chain so the PE
                        # is never head-of-line blocked on late W slabs
                        xt0 = xq.pop(0)
                        xt1 = xq.pop(1)
                        emit_qT(0, xt0, eightbank=True)
                        issue_x(2)
                        emit_qT(1, xt1, eightbank=False)
                        kps0, vps0 = emit_kv_mm(0, xt0)
                        sb0 = emit_epilogue(0, kps0, vps0)
                        issue_x(3)
                        kps1, vps1 = emit_kv_mm(1, xt1)
                        sb1 = emit_epilogue(1, kps1, vps1)
                        emit_kvbatch(1, 1, sb0, sb1, kv_acc)
                        start_i = 2
                    for i in range(start_i, NTG):
                        t = 8 * g + i
                        if i < NTG - 2:
                            issue_x(t + 2)
                        xt = xq.pop(t)
                        emit_qT(t, xt, eightbank=False)
                        kps, vps = emit_kv_mm(t, xt)
                        sb = emit_epilogue(t, kps, vps)
                        if i % 2 == 0:
                            hold = sb
                        else:
                            emit_kvbatch(i, t, hold, sb, kv_acc)
                            hold = None

                    arin = drampool.tile(
                        [128, NPAIR, KVW], F32, name=f"arin{g}", tag=f"arin{g}"
                    )
                    nc.sync.dma_start(arin[:], kv_acc[:])
                    arout = drampool.tile(
                        [128, NPAIR, KVW], F32, name=f"arout{g}", tag=f"arout{g}"
                    )
                    if sim_mode:
                        nc.sync.dma_start(arout[:], arin[:])
                    else:
                        nc.gpsimd.collective_compute(
                            "AllReduce",
                            ALU.add,
                            replica_groups=RG,
                            ins=[arin.opt()],
                            outs=[arout.opt()],
                        )
                    if g == 0:
                        prepped.append(kv_prep(g, arout))
                    else:
                        arout_g1 = arout
                    if g == 0:
                        w8o = []
                        for hl in range(2):
                            w = wpool.tile(
                                [128, 4, 2, DIM], FP8, name=f"wo8_{hl}",
                                tag=f"wo{hl}",
                            )
                            w8o.append(w)
                            nc.sync.dma_start(w[:], wout_d[hl, :, :, :, :])

            # ------------- phase 2 -------------
            with ExitStack() as ph2:
                atp = ph2.enter_context(tc.tile_pool(name="atp", bufs=1))
                recp = ph2.enter_context(tc.tile_pool(name="recp", bufs=3))
                osbp = ph2.enter_context(tc.tile_pool(name="osbp", bufs=3))
                psattn = ph2.enter_context(
                    tc.tile_pool(name="psattn", bufs=2, space="PSUM")
                )
                psnorm = ph2.enter_context(
                    tc.tile_pool(name="psnorm", bufs=2, space="PSUM")
                )
                psout = ph2.enter_context(
                    tc.tile_pool(name="psout", bufs=4, space="PSUM")
                )

                # outproj DoubleRow passes: (w_hl, attn_hl)
                OPASSES = ((0, 0), (0, 1), (1, 0))

                def emit_div(cc, bd, ksr, att2, c4s):
                    """aps/nps matmuls + reciprocal/mult (DVE) + fp8 hi
                    (ACT) + fp8 lo (Pool; DVE for the first two chunks so
                    the g=1 AllReduce sitting in Pool's FIFO cannot stall
                    them) for all 4 pair-pairs of chunk cc."""
                    c0 = cc * 256
                    for c4 in c4s:
                        aps = psattn.tile(
                            [128, 2, 256], F32, name=f"aps{cc}_{c4}", tag="aps"
                        )
                        nps = psnorm.tile(
                            [128, 2, 256], F32, name=f"nps{cc}_{c4}", tag="nps"
                        )
                        for i2 in range(2):
                            p = 2 * c4 + i2
                            nc.tensor.matmul(
                                aps[:, i2, :], bd[:, p, :],
                                qT[:, p, c0 : c0 + 256],
                            )
                            nc.tensor.matmul(
                                nps[:, i2, :], ksr[:, p, :],
                                qT[:, p, c0 : c0 + 256],
                            )
                        rec = recp.tile(
                            [128, 2, 256], F32, name=f"rec{cc}_{c4}", tag="rec"
                        )
                        nc.vector.reciprocal_approx_fast(rec[:], nps[:])
                        tmp = recp.tile(
                            [128, 2, 256], BF16, name=f"tmp{cc}_{c4}", tag="tmpb"
                        )
                        nc.vector.tensor_tensor(tmp[:], aps[:], rec[:], ALU.mult)
                        nc.scalar.activation(
                            att2[0][c4][:], tmp[:], ACTF.Copy, scale=1.0
                        )
                        # split the lo-subtracts between DVE and Pool: DVE
                        # fits two per chunk, and Pool's FIFO (which holds
                        # the g=1 AllReduce) never gates the early chunks.
                        sub_eng = (
                            nc.vector if (cc < 2 or c4 % 2 == 0) else nc.gpsimd
                        )
                        sub_eng.tensor_tensor(
                            att2[1][c4][:], tmp[:], att2[0][c4][:], ALU.subtract
                        )

                def emit_outproj(cc, tt, ops, att2, ffs=(0, 1), ffmajor=False):
                    """the 12-matmul DoubleRow chains for token half tt."""
                    tsl = slice(tt * 128, (tt + 1) * 128)
                    order = (
                        [(ff, c4) for ff in ffs for c4 in range(4)]
                        if ffmajor
                        else [(ff, c4) for c4 in range(4) for ff in ffs]
                    )
                    for ff, c4 in order:
                        fsl = slice(ff * 512, (ff + 1) * 512)
                        for wh, ah in OPASSES:
                            dr_mm(
                                ops[tt][ff][:],
                                att2[ah][c4][:, :, tsl],
                                w8o[wh][:, c4, :, fsl],
                                start=(c4 == 0 and wh == 0 and ah == 0),
                                stop=(c4 == 3 and wh == 1),
                            )

                def emit_osb(cc, tt, ops, osb=None, ffs=(0, 1)):
                    r0 = cc * 256 + tt * 128
                    if osb is None:
                        osb = osbp.tile(
                            [128, DIM], F32, name=f"osb{cc}{tt}", tag="osb"
                        )
                    for ff in ffs:
                        fsl = slice(ff * 512, (ff + 1) * 512)
                        if use_bias:
                            nc.vector.scalar_tensor_tensor(
                                out=osb[:, fsl],
                                in0=ops[tt][ff][:],
                                scalar=osc_sb[:],
                                in1=bout_sb[:, fsl],
                                op0=ALU.mult,
                                op1=ALU.add,
                            )
                        else:
                            nc.scalar.activation(
                                osb[:, fsl], ops[tt][ff][:],
                                ACTF.Copy, scale=OSCALE,
                            )
                        (nc.sync if cc == 7 else nc.scalar).dma_start(
                            out_d[r0 : r0 + 128, fsl], osb[:, fsl]
                        )

                prev = None  # (cc, ops, att2) whose tt1 chains are pending
                for cc in range(8):
                    if cc == 2:
                        # g=1's collective has had phase-1 tail + 2 cc of
                        # cover; prep now so Pool's FIFO never blocks the
                        # att2 splits on the AllReduce.
                        prepped.append(kv_prep(1, arout_g1))
                    bd, ksr = prepped[cc // 4]
                    att2 = [
                        [
                            atp.tile(
                                [128, 2, 256], FP8,
                                name=f"at{cc}_{hl}_{c4}",
                                tag=f"at{hl}_{c4}_{cc % 2}",
                            )
                            for c4 in range(4)
                        ]
                        for hl in range(2)
                    ]
                    ops = [
                        [
                            psout.tile(
                                [128, 512], F32, name=f"o{cc}_{tt}_{ff}",
                                tag="ops",
                            )
                            for ff in range(2)
                        ]
                        for tt in range(2)
                    ]
                    emit_div(cc, bd, ksr, att2, (0, 1))
                    if prev is not None:
                        pcc, pops, patt2 = prev
                        emit_outproj(pcc, 0, pops, patt2)
                        emit_osb(pcc, 0, pops)
                    emit_div(cc, bd, ksr, att2, (2, 3))
                    if prev is not None:
                        emit_outproj(pcc, 1, pops, patt2)
                        emit_osb(pcc, 1, pops)
                    prev = (cc, ops, att2)
                emit_outproj(7, 0, ops, att2)
                emit_osb(7, 0, ops)
                osb71 = osbp.tile([128, DIM], F32, name="osb71", tag="osb")
                emit_outproj(7, 1, ops, att2, ffs=(0,))
                emit_osb(7, 1, ops, osb=osb71, ffs=(0,))
                emit_outproj(7, 1, ops, att2, ffs=(1,))
                emit_osb(7, 1, ops, osb=osb71, ffs=(1,))

    nc.compile()
    return nc


def _get_nc(use_bias: bool, use_w: bool):
    key = ("nc", use_bias, use_w)
    if key not in _CACHE:
        _CACHE[key] = _build(use_bias, use_w)
    return _CACHE[key]


def _split8(a):
    np8 = mybir.dt.np(FP8)
    hi = a.astype(np8)
    lo = (a - hi.astype(np.float32)).astype(np8)
    return hi, lo


def make_in_maps(x, W_qkv, qn_w, kn_w, W_out, b_out):
    np8 = mybir.dt.np(FP8)
    x = np.asarray(x, dtype=np.float32)

    Ws = np.asarray(W_qkv, dtype=np.float32) * 32.0
    wh, wl = _split8(Ws)
    # [1024, 3072] -> [hl, c, p, i, m] = [2, 4, 128, 2, 3072]
    wq8 = np.ascontiguousarray(
        np.stack([wh, wl], axis=0)
        .reshape(2, 4, 2, 128, 3 * DIM)
        .transpose(0, 1, 3, 2, 4)
    )

    Wos = np.asarray(W_out, dtype=np.float32) * 32.0
    woh, wol = _split8(Wos)
    # [1024, 1024] -> [hl, p, c4, i, o] = [2, 128, 4, 2, 1024]
    wo8 = np.ascontiguousarray(
        np.stack([woh, wol], axis=0)
        .reshape(2, 4, 2, 128, DIM)
        .transpose(0, 3, 1, 2, 4)
    )

    qn = np.ascontiguousarray(
        np.asarray(qn_w, dtype=np.float32).reshape(8, 128).T
    )
    kn = np.ascontiguousarray(
        np.broadcast_to(np.asarray(kn_w, dtype=np.float32).reshape(1, DIM), (128, DIM))
    )
    bout = np.ascontiguousarray(
        np.broadcast_to(np.asarray(b_out, dtype=np.float32).reshape(1, DIM), (128, DIM))
    )
    in_maps = []
    for c in range(8):
        b0 = 2 * (c // 4)
        q = c % 4
        sl = slice(1024 * q, 1024 * (q + 1))
        xt = np.concatenate(
            [x[b0, sl, :].T, x[b0 + 1, sl, :].T], axis=1
        )  # [1024 dims, 2048 tokens]
        xh, xl = _split8(xt)
        # [hl, dim, tok] -> [t, p, hl, c', n]:
        # [2, 8(c'), 128(p), 16(t), 128(n)] -> transpose (3, 2, 0, 1, 4)
        x8 = np.ascontiguousarray(
            np.stack([xh, xl], axis=0)
            .reshape(2, 8, 128, 16, 128)
            .transpose(3, 2, 0, 1, 4)
        )
        in_maps.append(
            {
                "xT": x8,
                "wqkv": wq8,
                "wout": wo8,
                "qn": qn,
                "kn": kn,
                "bout": bout,
            }
        )
    return in_maps


def assemble(results):
    out = np.empty((B, N, DIM), dtype=np.float32)
    for b in range(B):
        base = 4 * (b // 2)
        g = b % 2
        for q in range(4):
            out[b, 1024 * q : 1024 * (q + 1), :] = results[base + q]["out"][
                1024 * g : 1024 * (g + 1), :
            ]
    return out


def run(in_maps, use_bias, use_w, **kw):
    nc = _get_nc(use_bias, use_w)
    return run_bass_kernel_spmd(nc, in_maps, core_ids=list(range(8)), **kw)


def kernel(x, W_qkv, qn_w, kn_w, W_out, b_out):
    use_bias = bool(np.any(np.asarray(b_out)))
    use_w = not (
        np.all(np.asarray(qn_w) == 1.0) and np.all(np.asarray(kn_w) == 1.0)
    )
    in_maps = make_in_maps(x, W_qkv, qn_w, kn_w, W_out, b_out)
    res = run(in_maps, use_bias, use_w)
    return assemble(res.results)


# revision 13
# speedup vs baseline: 1.0016x; 1.0016x over previous
"""Linear-attention (relu, rmsnorm-qk) Trainium2 Bass kernel, 8 NeuronCores.

Sharding: each core owns 1/4 of the tokens of TWO batch elements:
  cores 0-3 -> batches 0 (group g=0) and 1 (g=1)
  cores 4-7 -> batches 2 (g=0) and 3 (g=1)
Within a batch, core q (= core_id % 4) owns tokens [1024*q, 1024*(q+1)).

v3 design notes (fp8 DoubleRow with hi+lo error compensation):
 - The two big GEMMs (qkv projection, output projection) run as fp8e4m3
   DoubleRow matmuls (2 contraction rows per partition, 0.5 cyc/row).
   Full accuracy is kept by splitting each operand X into
   X_hi = fp8(X), X_lo = fp8(X - X_hi) and accumulating three passes
   hi@hi + lo@hi + hi@lo in PSUM (the dropped lo@lo term is ~0.1%).
   Net cost 6 cyc/col vs bf16's 8, with bf16-level accuracy.
 - W_qkv/W_out are pre-scaled by 32 (sigma 1) so fp8 normals cover them;
   the 32s cancel between attention numerator/denominator, and the
   output projection applies 1/(32*alpha) at the final copy.
 - attn is produced at alpha=128 times its true scale (folded into the
   v copy-out scale) so its fp8 hi/lo split lands in e4m3's normal
   range; split runs on the otherwise-idle Pool engine.
 - q's rmsnorm scale cancels entirely (relu(s*q) = s*relu(q)); k's
   scale is folded into v and the appended ones column, with stats
   taken on the 32-scaled k (eps' = 1024*eps).
 - kv_ext = k~^T @ [v~ | s] per head-pair in PSUM over 2-tile batches,
   AllReduce over the 4 cores of the batch overlaps the other group's
   phase 1; phase 2 attn^T = blockdiag(kv)^T @ q^T stays bf16 (K=128
   per pair: DoubleRow would not help).
"""

import os
import sys

import numpy as np

for _p in ("/opt/trn_rl_repo",):
    if _p not in sys.path and os.path.isdir(_p):
        sys.path.insert(0, _p)

import concourse.mybir as mybir
import concourse.tile as tile
from concourse import bacc
from concourse.bass_utils import run_bass_kernel_spmd
from contextlib import ExitStack

F32 = mybir.dt.float32
BF16 = mybir.dt.bfloat16
FP8 = mybir.dt.float8e4
DRMODE = mybir.MatmulPerfMode.DoubleRow
ALU = mybir.AluOpType
ACTF = mybir.ActivationFunctionType

DIM = 1024
HEADS = 16
DHEAD = 64
NPAIR = HEADS // 2          # 8 head pairs
B = 4
N = 4096
TOK = 2048                  # tokens per core (2 groups x 1024)
GTOK = 1024                 # tokens per group
NTG = GTOK // 128           # 8 token tiles per group
EPS_NORM = 1e-6
EPSP = 1024.0 * EPS_NORM    # stats run on 32x-scaled k
ALPHA = 128.0               # attn pre-scale for fp8 range
OSCALE = 1.0 / (32.0 * ALPHA)
KVW = 2 * (DHEAD + 1)       # 130: kv_ext width per pair
RG = [[0, 1, 2, 3], [4, 5, 6, 7]]

_CACHE: dict = {}


def _build(use_bias: bool, use_w: bool, sim_mode: bool = False):
    ndev = 1 if sim_mode else 8
    nc = bacc.Bacc("TRN2", target_bir_lowering=False, debug=False, num_devices=ndev)

    # x^T per token tile: [t, p, hl, c', n] = hi/lo of x[tok 128t+n, dim 128c'+p]
    xT_d = nc.dram_tensor("xT", [16, 128, 2, 8, 128], FP8, kind="ExternalInput").ap()
    # W_qkv*32 packed for DoubleRow: [hl, c, p, i, m] = 32*W[256c+128i+p, m]
    wqkv_d = nc.dram_tensor(
        "wqkv", [2, 4, 128, 2, 3 * DIM], FP8, kind="ExternalInput"
    ).ap()
    # W_out*32 packed: [hl, p, c4, i, o] = 32*Wout[256c4+128i+p, o]
    wout_d = nc.dram_tensor(
        "wout", [2, 128, 4, 2, DIM], FP8, kind="ExternalInput"
    ).ap()
    qn_d = nc.dram_tensor("qn", [128, 8], F32, kind="ExternalInput").ap()
    kn_d = nc.dram_tensor("kn", [128, DIM], F32, kind="ExternalInput").ap()
    bout_d = nc.dram_tensor("bout", [128, DIM], F32, kind="ExternalInput").ap()
    out_d = nc.dram_tensor("out", [TOK, DIM], F32, kind="ExternalOutput").ap()

    with tile.TileContext(nc) as tc:
        with ExitStack() as outer:
            const = outer.enter_context(tc.tile_pool(name="const", bufs=1))
            wpool = outer.enter_context(tc.tile_pool(name="wpool", bufs=1))
            qTpool = outer.enter_context(tc.tile_pool(name="qTpool", bufs=1))
            stats = outer.enter_context(tc.tile_pool(name="stats", bufs=3))
            drampool = outer.enter_context(
                tc.tile_pool(name="dram", bufs=1, space="DRAM")
            )

            eps_sb = const.tile([128, 1], F32, name="eps_sb")
            nc.vector.memset(eps_sb[:], EPSP)
            eps16_sb = const.tile([128, 1], F32, name="eps16_sb")
            nc.vector.memset(eps16_sb[:], EPSP / 16.0)
            ones_sb = const.tile([128, 64], F32, name="ones_sb")
            nc.vector.memset(ones_sb[:], 1.0)
            if use_w:
                qn_sb = const.tile([128, 8], F32, name="qn_sb")
                kn_sb = const.tile([128, DIM], F32, name="kn_sb")
                nc.sync.dma_start(qn_sb[:], qn_d[:])
                nc.sync.dma_start(kn_sb[:], kn_d[:])
            if use_bias:
                bout_sb = const.tile([128, DIM], F32, name="bout_sb")
                nc.sync.dma_start(bout_sb[:], bout_d[:])
                osc_sb = const.tile([128, 1], F32, name="osc_sb")
                nc.vector.memset(osc_sb[:], OSCALE)

            # x tiles stream on the ACT queue; tile 0 is issued before W so
            # its transfer leads the serial DMA stream.
            xTp = outer.enter_context(tc.tile_pool(name="xTp", bufs=5))
            xq = {}

            def issue_x(t):
                xt = xTp.tile([128, 2, 8, 128], FP8, name=f"xT_{t}", tag="xT")
                nc.scalar.dma_start(xt[:], xT_d[t, :, :, :, :])
                xq[t] = xt

            issue_x(0)

            # W_qkv resident as fp8 hi/lo 256-dim chunks. DMA engine pool
            # drains roughly in issue order: q columns (hi then lo, consumed
            # by tile 0's q^T chains) before the k/v columns.
            w8 = [[None] * 4 for _ in range(2)]
            for hl in range(2):
                for c in range(4):
                    w = wpool.tile(
                        [128, 2, 3 * DIM], FP8, name=f"w8_{hl}_{c}",
                        tag=f"w{hl}{c}",
                    )
                    w8[hl][c] = w
                    nc.sync.dma_start(
                        w[:, :, 0:DIM], wqkv_d[hl, c, :, :, 0:DIM]
                    )
            # k then v slabs, hi before lo: exact first-use order so the
            # PE consumes each 256-dim chunk the moment it lands.
            for base in (DIM, 2 * DIM):
                for hl in range(2):
                    for c in range(4):
                        nc.sync.dma_start(
                            w8[hl][c][:, :, base : base + DIM],
                            wqkv_d[hl, c, :, :, base : base + DIM],
                        )

            # all 8 pairs' q^T in one tile: [128, pair, TOK]
            qT = qTpool.tile([128, NPAIR, TOK], BF16, name="qT", tag="qT")
            prep = outer.enter_context(tc.tile_pool(name="prep", bufs=1))

            def kv_prep(g, arout):
                # Entirely on the gpsimd queue: it is otherwise idle-ish, its
                # FIFO orders the load after the collective.
                kv_sb = prep.tile(
                    [128, NPAIR, KVW], F32, name=f"kvsb{g}", tag=f"kvsb{g}"
                )
                nc.gpsimd.dma_start(kv_sb[:], arout[:])
                bd = prep.tile([128, NPAIR, 128], BF16, name=f"bd{g}", tag=f"bd{g}")
                nc.gpsimd.memset(bd[:], 0.0)
                nc.gpsimd.tensor_copy(bd[0:64, :, 0:64], kv_sb[0:64, :, 0:64])
                nc.gpsimd.tensor_copy(
                    bd[64:128, :, 64:128], kv_sb[64:128, :, 65:129]
                )
                ksr = prep.tile(
                    [128, NPAIR, 128], BF16, name=f"ksr{g}", tag=f"ksr{g}"
                )
                nc.gpsimd.memset(ksr[:], 0.0)
                for p in range(NPAIR):
                    nc.gpsimd.tensor_scalar_mul(
                        ksr[0:64, p, 0:64], ones_sb[0:64, 0:64],
                        kv_sb[0:64, p, 64:65],
                    )
                    nc.gpsimd.tensor_scalar_mul(
                        ksr[64:128, p, 64:128], ones_sb[64:128, 0:64],
                        kv_sb[64:128, p, 129:130],
                    )
                return bd, ksr

            def dr_mm(ps, lhsT, rhs, start, stop):
                nc.tensor.matmul(
                    ps, lhsT, rhs, start=start, stop=stop, perf_mode=DRMODE
                )

            prepped = []
            kp = outer.enter_context(tc.tile_pool(name="kp", bufs=4))
            vp = outer.enter_context(tc.tile_pool(name="vp", bufs=4))
            kvpool = outer.enter_context(tc.tile_pool(name="kvpool", bufs=2))
            with ExitStack() as ph1:
                psq = ph1.enter_context(
                    tc.tile_pool(name="psq", bufs=1, space="PSUM")
                )
                psk = ph1.enter_context(
                    tc.tile_pool(name="psk", bufs=1, space="PSUM")
                )
                psv = ph1.enter_context(
                    tc.tile_pool(name="psv", bufs=1, space="PSUM")
                )
                pssm = ph1.enter_context(
                    tc.tile_pool(name="pssm", bufs=1, space="PSUM")
                )

                # DoubleRow pass list for the qkv GEMM: (w_hl, x_hl)
                PASSES = ((0, 0), (0, 1), (1, 0))

                def emit_qT(t, xt, eightbank):
                    t0r = t * 128
                    if eightbank:
                        # tile 0: all 8 banks free -> 8 chains c-major (one
                        # per bank), consuming each W q-part as it lands.
                        tags = ("q0", "q1", "k0", "k1", "v0", "v1", "sm0", "sm1")
                        pools = (psq, psq, psk, psk, psv, psv, pssm, pssm)
                        q8 = [
                            pools[j].tile(
                                [128, 128], F32, name=f"q8_{j}", tag=tags[j]
                            )
                            for j in range(8)
                        ]
                        for wh, xh in PASSES:
                            for c in range(4):
                                for j in range(8):
                                    dr_mm(
                                        q8[j][:],
                                        w8[wh][c][:, :, j * 128 : (j + 1) * 128],
                                        xt[:, xh, 2 * c : 2 * c + 2, :],
                                        start=(wh == 0 and xh == 0 and c == 0),
                                        stop=(wh == 1 and c == 3),
                                    )
                        for j in range(8):
                            nc.scalar.activation(
                                qT[:, j, t0r : t0r + 128],
                                q8[j][:],
                                ACTF.Relu,
                                scale=(qn_sb[:, j : j + 1] if use_w else 1.0),
                            )
                        return
                    qps = [
                        psq.tile(
                            [128, 4, 128], F32,
                            name=f"qps{t}_{half}", tag=f"q{half}",
                        )
                        for half in range(2)
                    ]
                    for half in range(2):
                        for jj in range(4):
                            j = 4 * half + jj
                            for pi, (wh, xh) in enumerate(PASSES):
                                for c in range(4):
                                    dr_mm(
                                        qps[half][:, jj, :],
                                        w8[wh][c][:, :, j * 128 : (j + 1) * 128],
                                        xt[:, xh, 2 * c : 2 * c + 2, :],
                                        start=(pi == 0 and c == 0),
                                        stop=(pi == 2 and c == 3),
                                    )
                        if use_w:
                            for jj in range(4):
                                j = 4 * half + jj
                                nc.scalar.activation(
                                    qT[:, j, t0r : t0r + 128],
                                    qps[half][:, jj, :],
                                    ACTF.Relu,
                                    scale=qn_sb[:, j : j + 1],
                                )
                        else:
                            nc.scalar.activation(
                                qT[:, 4 * half : 4 * half + 4, t0r : t0r + 128],
                                qps[half][:],
                                ACTF.Relu,
                                scale=1.0,
                            )

                def emit_kv_mm(t, xt):
                    kps = [
                        psk.tile([128, 512], F32, name=f"kps{t}_{h}", tag=f"k{h}")
                        for h in range(2)
                    ]
                    vps = [
                        psv.tile([128, 512], F32, name=f"vps{t}_{h}", tag=f"v{h}")
                        for h in range(2)
                    ]

                    def kv_chunk(ps, base, h, c, wh, xh, start, stop):
                        dr_mm(
                            ps[h][:],
                            xt[:, xh, 2 * c : 2 * c + 2, :],
                            w8[wh][c][
                                :, :, base + h * 512 : base + (h + 1) * 512
                            ],
                            start=start,
                            stop=stop,
                        )

                    for h in range(2):
                        for pi, (wh, xh) in enumerate(PASSES):
                            for c in range(4):
                                kv_chunk(
                                    kps, DIM, h, c, wh, xh,
                                    (pi == 0 and c == 0),
                                    (pi == 2 and c == 3),
                                )
                    for h in range(2):
                        for pi, (wh, xh) in enumerate(PASSES):
                            for c in range(4):
                                kv_chunk(
                                    vps, 2 * DIM, h, c, wh, xh,
                                    (pi == 0 and c == 0),
                                    (pi == 2 and c == 3),
                                )
                    return kps, vps

                def emit_epilogue(t, kps, vps):
                    # rmsnorm scale on 32x-scaled k: s = 1/sqrt(ms + 1024eps)
                    # (= s_true/32, the ones-column scale); v copy scale is
                    # 4x that (alpha=128 folded: alpha*s_true/1024).
                    st6 = stats.tile([128, 2, 6], F32, name=f"st6_{t}", tag="st6")
                    nc.vector.bn_stats(st6[:, 0, :], kps[0][:])
                    nc.vector.bn_stats(st6[:, 1, :], kps[1][:])
                    mv = stats.tile([128, 2], F32, name=f"mv_{t}", tag="mv")
                    nc.vector.bn_aggr(mv[:], st6[:])
                    ms = stats.tile([128, 1], F32, name=f"ms_{t}", tag="ms")
                    nc.vector.scalar_tensor_tensor(
                        out=ms[:],
                        in0=mv[:, 0:1],
                        scalar=mv[:, 0:1],
                        in1=mv[:, 1:2],
                        op0=ALU.mult,
                        op1=ALU.add,
                    )
                    a0 = stats.tile([128, 1], F32, name=f"a0_{t}", tag="a0")
                    nc.scalar.activation(
                        a0[:], ms[:], ACTF.Sqrt, bias=eps_sb[:], scale=1.0
                    )
                    s = stats.tile([128, 1], F32, name=f"s_{t}", tag="s")
                    nc.vector.reciprocal(s[:], a0[:])
                    a0v = stats.tile([128, 1], F32, name=f"a0v_{t}", tag="a0v")
                    nc.scalar.activation(
                        a0v[:], ms[:], ACTF.Sqrt, bias=eps16_sb[:], scale=0.0625
                    )
                    sv = stats.tile([128, 1], F32, name=f"sv_{t}", tag="sv")
                    nc.vector.reciprocal(sv[:], a0v[:])

                    # k~ = relu(k) (scale folded into v); general path
                    # applies kn first on DVE.
                    k_sb = kp.tile([128, DIM], BF16, name=f"ksb{t}", tag="ksb")
                    for h in range(2):
                        sl = slice(h * 512, (h + 1) * 512)
                        if use_w:
                            nc.vector.tensor_tensor(
                                k_sb[:, sl], kps[h][:], kn_sb[:, sl], ALU.mult
                            )
                            nc.scalar.activation(
                                k_sb[:, sl], k_sb[:, sl], ACTF.Relu
                            )
                        else:
                            nc.scalar.activation(
                                k_sb[:, sl], kps[h][:], ACTF.Relu
                            )

                    v_sb = vp.tile(
                        [128, HEADS, DHEAD + 1], BF16, name=f"vsb{t}", tag="vsb"
                    )
                    for h in range(2):
                        nc.scalar.activation(
                            v_sb[:, 8 * h : 8 * (h + 1), 0:DHEAD],
                            vps[h].rearrange("p (h e) -> p h e", e=DHEAD),
                            ACTF.Copy,
                            scale=sv[:],
                        )
                    nc.vector.tensor_scalar_mul(
                        v_sb[:, :, DHEAD], ones_sb[:, 0:16], s[:]
                    )
                    return k_sb, v_sb

                def emit_kvbatch(i, t, sb0, sb1, kv_acc):
                    pk, pv = sb0
                    k_sb, v_sb = sb1
                    for grp, prs in ((0, (0, 1, 2)), (1, (3, 4, 5)), (2, (6, 7))):
                        kvp = pssm.tile(
                            [128, len(prs), KVW], F32,
                            name=f"kv{t}_{grp}",
                            tag=f"sm{0 if grp != 1 else 1}",
                        )
                        for pi, p in enumerate(prs):
                            for ii, (ks_, vs_) in enumerate(
                                ((pk, pv), (k_sb, v_sb))
                            ):
                                nc.tensor.matmul(
                                    kvp[:, pi, :],
                                    ks_[:, p * 128 : (p + 1) * 128],
                                    vs_[:, 2 * p : 2 * p + 2, :],
                                    start=(ii == 0),
                                    stop=(ii == 1),
                                )
                        if i == 1:
                            nc.vector.tensor_copy(
                                kv_acc[:, prs[0] : prs[-1] + 1, :], kvp[:]
                            )
                        else:
                            nc.vector.tensor_add(
                                kv_acc[:, prs[0] : prs[-1] + 1, :],
                                kv_acc[:, prs[0] : prs[-1] + 1, :],
                                kvp[:],
                            )

                for g in range(2):
                    kv_acc = kvpool.tile(
                        [128, NPAIR, KVW], F32, name=f"kvacc{g}", tag="kvacc"
                    )
                    if g > 0:
                        issue_x(8 * g)
                    issue_x(8 * g + 1)
                    hold = None
                    start_i = 0
                    if g == 0:
                        # prologue: both tiles' q^T (gated only on the early
                        # q-part stream) run before any k/v chain so the PE
                        # is never head-of-line blocked on late W slabs
                        xt0 = xq.pop(0)
                        xt1 = xq.pop(1)
                        emit_qT(0, xt0, eightbank=True)
                        issue_x(2)
                        emit_qT(1, xt1, eightbank=False)
                        kps0, vps0 = emit_kv_mm(0, xt0)
                        sb0 = emit_epilogue(0, kps0, vps0)
                        issue_x(3)
                        kps1, vps1 = emit_kv_mm(1, xt1)
                        sb1 = emit_epilogue(1, kps1, vps1)
                        emit_kvbatch(1, 1, sb0, sb1, kv_acc)
                        start_i = 2
                    for i in range(start_i, NTG):
                        t = 8 * g + i
                        if i < NTG - 2:
                            issue_x(t + 2)
                        xt = xq.pop(t)
                        emit_qT(t, xt, eightbank=False)
                        kps, vps = emit_kv_mm(t, xt)
                        sb = emit_epilogue(t, kps, vps)
                        if i % 2 == 0:
                            hold = sb
                        else:
                            emit_kvbatch(i, t, hold, sb, kv_acc)
                            hold = None

                    arin = drampool.tile(
                        [128, NPAIR, KVW], F32, name=f"arin{g}", tag=f"arin{g}"
                    )
                    nc.sync.dma_start(arin[:], kv_acc[:])
                    arout = drampool.tile(
                        [128, NPAIR, KVW], F32, name=f"arout{g}", tag=f"arout{g}"
                    )
                    if sim_mode:
                        nc.sync.dma_start(arout[:], arin[:])
                    else:
                        nc.gpsimd.collective_compute(
                            "AllReduce",
                            ALU.add,
                            replica_groups=RG,
                            ins=[arin.opt()],
                            outs=[arout.opt()],
                        )
                    if g == 0:
                        prepped.append(kv_prep(g, arout))
                    else:
                        arout_g1 = arout
                    if g == 0:
                        w8o = []
                        for hl in range(2):
                            w = wpool.tile(
                                [128, 4, 2, DIM], FP8, name=f"wo8_{hl}",
                                tag=f"wo{hl}",
                            )
                            w8o.append(w)
                            nc.sync.dma_start(w[:], wout_d[hl, :, :, :, :])

            # ------------- phase 2 -------------
            with ExitStack() as ph2:
                atp = ph2.enter_context(tc.tile_pool(name="atp", bufs=1))
                recp = ph2.enter_context(tc.tile_pool(name="recp", bufs=3))
                osbp = ph2.enter_context(tc.tile_pool(name="osbp", bufs=3))
                psattn = ph2.enter_context(
                    tc.tile_pool(name="psattn", bufs=2, space="PSUM")
                )
                psnorm = ph2.enter_context(
                    tc.tile_pool(name="psnorm", bufs=2, space="PSUM")
                )
                psout = ph2.enter_context(
                    tc.tile_pool(name="psout", bufs=4, space="PSUM")
                )

                # outproj DoubleRow passes: (w_hl, attn_hl)
                OPASSES = ((0, 0), (0, 1), (1, 0))

                def emit_div(cc, bd, ksr, att2, c4s):
                    """aps/nps matmuls + reciprocal/mult (DVE) + fp8 hi
                    (ACT) + fp8 lo (Pool; DVE for the first two chunks so
                    the g=1 AllReduce sitting in Pool's FIFO cannot stall
                    them) for all 4 pair-pairs of chunk cc."""
                    c0 = cc * 256
                    for c4 in c4s:
                        aps = psattn.tile(
                            [128, 2, 256], F32, name=f"aps{cc}_{c4}", tag="aps"
                        )
                        nps = psnorm.tile(
                            [128, 2, 256], F32, name=f"nps{cc}_{c4}", tag="nps"
                        )
                        for i2 in range(2):
                            p = 2 * c4 + i2
                            nc.tensor.matmul(
                                aps[:, i2, :], bd[:, p, :],
                                qT[:, p, c0 : c0 + 256],
                            )
                            nc.tensor.matmul(
                                nps[:, i2, :], ksr[:, p, :],
                                qT[:, p, c0 : c0 + 256],
                            )
                        rec = recp.tile(
                            [128, 2, 256], F32, name=f"rec{cc}_{c4}", tag="rec"
                        )
                        nc.vector.reciprocal_approx_fast(rec[:], nps[:])
                        tmp = recp.tile(
                            [128, 2, 256], BF16, name=f"tmp{cc}_{c4}", tag="tmpb"
                        )
                        nc.vector.tensor_tensor(tmp[:], aps[:], rec[:], ALU.mult)
                        nc.scalar.activation(
                            att2[0][c4][:], tmp[:], ACTF.Copy, scale=1.0
                        )
                        # split the lo-subtracts between DVE and Pool: DVE
                        # fits two per chunk, and Pool's FIFO (which holds
                        # the g=1 AllReduce) never gates the early chunks.
                        sub_eng = nc.vector if cc < 2 else nc.gpsimd
                        sub_eng.tensor_tensor(
                            att2[1][c4][:], tmp[:], att2[0][c4][:], ALU.subtract
                        )

                def emit_outproj(cc, tt, ops, att2, ffs=(0, 1), ffmajor=False):
                    """the 12-matmul DoubleRow chains for token half tt."""
                    tsl = slice(tt * 128, (tt + 1) * 128)
                    order = (
                        [(ff, c4) for ff in ffs for c4 in range(4)]
                        if ffmajor
                        else [(ff, c4) for c4 in range(4) for ff in ffs]
                    )
                    for ff, c4 in order:
                        fsl = slice(ff * 512, (ff + 1) * 512)
                        for wh, ah in OPASSES:
                            dr_mm(
                                ops[tt][ff][:],
                                att2[ah][c4][:, :, tsl],
                                w8o[wh][:, c4, :, fsl],
                                start=(c4 == 0 and wh == 0 and ah == 0),
                                stop=(c4 == 3 and wh == 1),
                            )

                def emit_osb(cc, tt, ops, osb=None, ffs=(0, 1)):
                    r0 = cc * 256 + tt * 128
                    if osb is None:
                        osb = osbp.tile(
                            [128, DIM], F32, name=f"osb{cc}{tt}", tag="osb"
                        )
                    for ff in ffs:
                        fsl = slice(ff * 512, (ff + 1) * 512)
                        if use_bias:
                            nc.vector.scalar_tensor_tensor(
                                out=osb[:, fsl],
                                in0=ops[tt][ff][:],
                                scalar=osc_sb[:],
                                in1=bout_sb[:, fsl],
                                op0=ALU.mult,
                                op1=ALU.add,
                            )
                        else:
                            nc.scalar.activation(
                                osb[:, fsl], ops[tt][ff][:],
                                ACTF.Copy, scale=OSCALE,
                            )
                        (nc.sync if cc == 7 else nc.scalar).dma_start(
                            out_d[r0 : r0 + 128, fsl], osb[:, fsl]
                        )

                prev = None  # (cc, ops, att2) whose tt1 chains are pending
                for cc in range(8):
                    if cc == 2:
                        # g=1's collective has had phase-1 tail + 2 cc of
                        # cover; prep now so Pool's FIFO never blocks the
                        # att2 splits on the AllReduce.
                        prepped.append(kv_prep(1, arout_g1))
                    bd, ksr = prepped[cc // 4]
                    att2 = [
                        [
                            atp.tile(
                                [128, 2, 256], FP8,
                                name=f"at{cc}_{hl}_{c4}",
                                tag=f"at{hl}_{c4}_{cc % 2}",
                            )
                            for c4 in range(4)
                        ]
                        for hl in range(2)
                    ]
                    ops = [
                        [
                            psout.tile(
                                [128, 512], F32, name=f"o{cc}_{tt}_{ff}",
                                tag="ops",
                            )
                            for ff in range(2)
                        ]
                        for tt in range(2)
                    ]
                    emit_div(cc, bd, ksr, att2, (0, 1))
                    if prev is not None:
                        pcc, pops, patt2 = prev
                        emit_outproj(pcc, 0, pops, patt2)
                        emit_osb(pcc, 0, pops)
                    emit_div(cc, bd, ksr, att2, (2, 3))
                    if prev is not None:
                        emit_outproj(pcc, 1, pops, patt2)
                        emit_osb(pcc, 1, pops)
                    prev = (cc, ops, att2)
                emit_outproj(7, 0, ops, att2)
                emit_osb(7, 0, ops)
                osb71 = osbp.tile([128, DIM], F32, name="osb71", tag="osb")
                emit_outproj(7, 1, ops, att2, ffs=(0,))
                emit_osb(7, 1, ops, osb=osb71, ffs=(0,))
                emit_outproj(7, 1, ops, att2, ffs=(1,))
                emit_osb(7, 1, ops, osb=osb71, ffs=(1,))

    nc.compile()
    return nc


def _get_nc(use_bias: bool, use_w: bool):
    key = ("nc", use_bias, use_w)
    if key not in _CACHE:
        _CACHE[key] = _build(use_bias, use_w)
    return _CACHE[key]


def _split8(a):
    np8 = mybir.dt.np(FP8)
    hi = a.astype(np8)
    lo = (a - hi.astype(np.float32)).astype(np8)
    return hi, lo


def make_in_maps(x, W_qkv, qn_w, kn_w, W_out, b_out):
    np8 = mybir.dt.np(FP8)
    x = np.asarray(x, dtype=np.float32)

    Ws = np.asarray(W_qkv, dtype=np.float32) * 32.0
    wh, wl = _split8(Ws)
    # [1024, 3072] -> [hl, c, p, i, m] = [2, 4, 128, 2, 3072]
    wq8 = np.ascontiguousarray(
        np.stack([wh, wl], axis=0)
        .reshape(2, 4, 2, 128, 3 * DIM)
        .transpose(0, 1, 3, 2, 4)
    )

    Wos = np.asarray(W_out, dtype=np.float32) * 32.0
    woh, wol = _split8(Wos)
    # [1024, 1024] -> [hl, p, c4, i, o] = [2, 128, 4, 2, 1024]
    wo8 = np.ascontiguousarray(
        np.stack([woh, wol], axis=0)
        .reshape(2, 4, 2, 128, DIM)
        .transpose(0, 3, 1, 2, 4)
    )

    qn = np.ascontiguousarray(
        np.asarray(qn_w, dtype=np.float32).reshape(8, 128).T
    )
    kn = np.ascontiguousarray(
        np.broadcast_to(np.asarray(kn_w, dtype=np.float32).reshape(1, DIM), (128, DIM))
    )
    bout = np.ascontiguousarray(
        np.broadcast_to(np.asarray(b_out, dtype=np.float32).reshape(1, DIM), (128, DIM))
    )
    in_maps = []
    for c in range(8):
        b0 = 2 * (c // 4)
        q = c % 4
        sl = slice(1024 * q, 1024 * (q + 1))
        xt = np.concatenate(
            [x[b0, sl, :].T, x[b0 + 1, sl, :].T], axis=1
        )  # [1024 dims, 2048 tokens]
        xh, xl = _split8(xt)
        # [hl, dim, tok] -> [t, p, hl, c', n]:
        # [2, 8(c'), 128(p), 16(t), 128(n)] -> transpose (3, 2, 0, 1, 4)
        x8 = np.ascontiguousarray(
            np.stack([xh, xl], axis=0)
            .reshape(2, 8, 128, 16, 128)
            .transpose(3, 2, 0, 1, 4)
        )
        in_maps.append(
            {
                "xT": x8,
                "wqkv": wq8,
                "wout": wo8,
                "qn": qn,
                "kn": kn,
                "bout": bout,
            }
        )
    return in_maps


def assemble(results):
    out = np.empty((B, N, DIM), dtype=np.float32)
    for b in range(B):
        base = 4 * (b // 2)
        g = b % 2
        for q in range(4):
            out[b, 1024 * q : 1024 * (q + 1), :] = results[base + q]["out"][
                1024 * g : 1024 * (g + 1), :
            ]
    return out


def run(in_maps, use_bias, use_w, **kw):
    nc = _get_nc(use_bias, use_w)
    return run_bass_kernel_spmd(nc, in_maps, core_ids=list(range(8)), **kw)


def kernel(x, W_qkv, qn_w, kn_w, W_out, b_out):
    use_bias = bool(np.any(np.asarray(b_out)))
    use_w = not (
        np.all(np.asarray(qn_w) == 1.0) and np.all(np.asarray(kn_w) == 1.0)
    )
    in_maps = make_in_maps(x, W_qkv, qn_w, kn_w, W_out, b_out)
    res = run(in_maps, use_bias, use_w)
    return assemble(res.results)
